# revision 30
# baseline (speedup 1.0000x reference)
"""ButterFlyNet2D forward on 8 trn2 NeuronCores (v3).

Sharding: core c handles layer-1 parent block (u0,v0) = divmod(c//2, 2) and
m-half ly = c%2 (2 of 16 layer-2 subtrees), full batch. Butterfly weights are
read exactly once across the 8 cores.

Numerics: single-pass bf16 matmuls everywhere (fp32 PSUM accumulate).
Measured rel err vs fp32 reference ~7e-3 (gate is 2e-2).

Activation layout (the key trick): act[l] tile columns use a bit-reversed
parity-major position ordering, col = f_r(Y,X)*32 + b with
f_r(Y,X) = sum_i (Y_i*2 + X_i) * 4^(r-1-i)  (LSB parity pair most
significant). With this ordering every PSUM->SBUF relu-scatter between
butterfly levels is a dense contiguous copy (the (p,q) patch parities of the
next layer are the top two bits), instead of a stride-2 gather. Partition
layout of act tiles: [128 = 64*q + n]; the y-parity p picks one of two tile
sets; the first layer K-stacks the two q-halves so its psum is scatter-free.

Schedule: 8 fills of [128,1024] PSUM per layer on a 4-buffer ring (2 PSUM
banks each) so tensor runs ~3 fills ahead of the relu-scatter. Relu-copies
are split vector/scalar with exactly one writer engine per act tile (mixed
writers create cross-engine WAW semaphore chains that serialize the copy
engines). Layers 4/5 matmuls are column-tiled into M=64 pairs (concurrent
col-group LDWEIGHTS halve the serial weight-load chain, the binding
constraint at free-dim 128/32). The final 1x1 layer runs 4 units per matmul
with a block-diagonal Wf (off-diagonal blocks computed and ignored) and
ships raw PSUM values; relu is applied on the host in decode.

Weights are packed on the host as [128 = 64q+n, p*2K + m*128 + k] bf16
blocks, grouped 8 blocks per DMA (8KB/partition lines), streamed in
consumption order on the sync queue behind w0+patches; layer-5 weights
prefetch through a 7-deep ring (~7MB ahead).
"""

import numpy as np
import ml_dtypes

# ---------------------------------------------------------------- constants
B, C, H, W, L, T = 32, 1, 64, 64, 6, 4
NCH = 64
KO = 256
N_CORES = 8
TILE_W = 2048
FILL_W = 1024

BF16 = ml_dtypes.bfloat16

# per layer l=1..5: input-block half-grid s2, cols per unit, #blocks
LAYER_S2 = {1: 16, 2: 8, 3: 4, 4: 2, 5: 1}
LAYER_NPOS = {l: 32 * LAYER_S2[l] ** 2 for l in LAYER_S2}   # 8192,2048,512,128,32
LAYER_NB = {1: 1, 2: 2, 3: 8, 4: 32, 5: 128}
LAYER_K2 = {1: 128, 2: 256, 3: 256, 4: 256, 5: 256}
NTILES = {1: 4, 2: 2, 3: 2, 4: 2, 5: 2}   # act tiles per (p)
W_SLOT = {1: 512, 2: 512, 3: 512, 4: 128, 5: 32}
SPF = {l: FILL_W // W_SLOT[l] for l in W_SLOT}              # slots per fill
NFILL = 8   # every layer runs 8 fills of [128, 1024]
NFIRST = 16
WGRP = {1: 1, 2: 2, 3: 8, 4: 8, 5: 8}                       # blocks per DMA group
NGRP = {l: LAYER_NB[l] // WGRP[l] for l in WGRP}


def core_geom(c):
    P, ly = divmod(c, 2)
    u0, v0 = divmod(P, 2)
    blocks = {1: [(u0, v0)]}
    for l in range(2, 6):
        ms = [ly] if l - 1 == 1 else [0, 1]
        nxt = []
        for (u, v) in blocks[l - 1]:
            for a in ms:
                for bb in (0, 1):
                    nxt.append((2 * u + a, 2 * v + bb))
        blocks[l] = nxt
    return u0, v0, ly, blocks


# ------------------------------------------------- bit-reversed position order
def f_interleave(r, Y, X):
    """f = sum_i (Y_i*2 + X_i) * 4^(r-1-i); parity pair (LSBs) most significant."""
    f = np.zeros_like(np.asarray(Y))
    for i in range(r):
        f = f + (((Y >> i) & 1) * 2 + ((X >> i) & 1)) * (4 ** (r - 1 - i))
    return f


def f4_inverse():
    """-> (Yp, Xp) arrays [256] with f_4(Yp[f], Xp[f]) = f."""
    Y = np.arange(16)[:, None] * np.ones(16, np.int64)[None, :]
    X = np.ones(16, np.int64)[:, None] * np.arange(16)[None, :]
    f = f_interleave(4, Y, X).reshape(-1)
    Yp = np.zeros(256, np.int64)
    Xp = np.zeros(256, np.int64)
    Yp[f] = Y.reshape(-1)
    Xp[f] = X.reshape(-1)
    return Yp, Xp


# ---------------------------------------------------------------- host packing
def pack_patches(x):
    """-> [8, 16384] bf16. col = f*2048 + fb*32 + b for fill f = p*4 + cc,
    f4 = cc*64 + fb; rows = 4*q + 2*h + w carry pixel (2(2Y'+p)+h, 2(2X'+q)+w).
    """
    xbf = np.asarray(x[:, 0], dtype=np.float32).astype(BF16).astype(np.float32)
    Yp, Xp = f4_inverse()
    out = np.zeros((8, 16384), dtype=BF16)
    b_idx = np.arange(32)
    for q in range(2):
        for h in range(2):
            for w in range(2):
                r = 4 * q + 2 * h + w
                for p in range(2):
                    rows = 2 * (2 * Yp + p) + h          # [256]
                    cols = 2 * (2 * Xp + q) + w          # [256]
                    vals = xbf[b_idx[None, :], rows[:, None], cols[:, None]]
                    # vals [256 f4, 32 b] -> fills p*4 + cc
                    out[r, p * 8192:(p + 1) * 8192] = (
                        vals.reshape(-1).astype(BF16))
    return out


def pack_first(W0, u0, v0):
    """-> [8, 128] bf16: [0:4,0:64] = [4(2h+w), 64k] = W0 block; dup at
    [4:8, 64:128] for the q=1 K-stack."""
    koff = (u0 * 2 + v0) * 64
    w0e = np.asarray(W0[0, koff:koff + 64, 0], dtype=np.float32)  # [64, 2, 2]
    wt = w0e.transpose(1, 2, 0).reshape(4, 64)
    out = np.zeros((8, 128), dtype=BF16)
    out[0:4, 0:64] = wt.astype(BF16)
    out[4:8, 64:128] = wt.astype(BF16)
    return out


def pack_weights_layer(Wl, l, blocks_l, ly):
    """-> [ngrp, 128, wgrp*2*K2] bf16; block ib at group ib>>3, col slot
    (ib % wgrp)*2K2 + p*K2 + k; partition = 64*q + n."""
    K2 = LAYER_K2[l]
    nb = LAYER_NB[l]
    arr = np.zeros((nb, 128, 2 * K2), dtype=BF16)
    for i, (u, v) in enumerate(blocks_l):
        wb = np.asarray(Wl[0, u, v], dtype=np.float32)      # [256, 64, 2, 2]
        if l == 1:
            wb = wb[ly * 128:(ly + 1) * 128]
        wt = wb.transpose(3, 1, 2, 0).reshape(128, 2 * K2)  # (q,n) x (p,k)
        arr[i] = wt.astype(BF16)
    wgrp = WGRP[l]
    ngrp = NGRP[l]
    return (arr.reshape(ngrp, wgrp, 128, 2 * K2)
               .transpose(0, 2, 1, 3)
               .reshape(ngrp, 128, wgrp * 2 * K2))


def pack_wf(Wf, blocks5):
    """-> [128, 2048] bf16 block-diag pairs; slot idx: cols [8*idx, 8*idx+8),
    rows 0:64 = Wf(klx=0 block).T at cols 0:4, rows 64:128 = klx=1 at 4:8."""
    out = np.zeros((128, 2048), dtype=np.float32)
    for idx in range(256):
        ib, m = idx // 2, idx % 2
        u, v = blocks5[ib]
        for klx in range(2):
            wft = np.asarray(Wf[0, 2 * u + m, 2 * v + klx], np.float32)  # [4,64]
            out[klx * 64:(klx + 1) * 64,
                idx * 8 + klx * 4:idx * 8 + klx * 4 + 4] = wft.T
    return out.astype(BF16)


# ------------------------------------------------------- scatter descriptors
# copy = (sp, scol, dp, p2, g, dcol, sstride, dstride, nblk, width)
# src psum[sp:sp+64, scol + j*sstride : +width], j in [0, nblk)
# dst act[l+1][p2][g][dp:dp+64, dcol + j*dstride : +width]
def first_plan(F):
    f, h = F >> 1, F & 1
    return [(-1, 0, -1, f >> 2, f & 3, h * 1024, 0, 0, 1, 1024)]


def layer_plan(l, F):
    copies = []
    if l == 1:
        f, h = F >> 1, F & 1
        for si in range(2):
            sio = 2 * h + si
            for klx in range(2):
                copies.append((64 * klx, si * 512, 64 * (sio & 1), sio >> 1,
                               klx, f * 512, 0, 0, 1, 512))
    elif l == 2:
        u, h = F >> 1, F & 1
        ib, m = u >> 1, u & 1
        for si in range(2):
            chk = 2 * h + si
            for klx in range(2):
                ibc = 4 * ib + 2 * m + klx
                copies.append((64 * klx, si * 512, 64 * (chk & 1), chk >> 1,
                               ibc >> 2, (ibc & 3) * 512, 0, 0, 1, 512))
    elif l == 3:
        # fill F = unit ib=F, slots m=0,1; merged over m (stride 512/256)
        for klx in range(2):
            for pq in range(4):
                ibc0 = 4 * F + klx
                g = ibc0 >> 4
                copies.append((64 * klx, pq * 128,
                               64 * (pq & 1), pq >> 1, g,
                               ibc0 * 128 - g * 2048, 512, 256, 2, 128))
    elif l == 4:
        # fill F = ib in [4F, 4F+4) x m; slot j = 2*di+m has uniform
        # src stride 128 / dst stride 64 over j
        for klx in range(2):
            for pq in range(4):
                ibc0 = 16 * F + klx
                g = ibc0 >> 6
                copies.append((64 * klx, pq * 32,
                               64 * (pq & 1), pq >> 1, g,
                               ibc0 * 32 - g * 2048, 128, 64, 8, 32))
    else:
        raise AssertionError(l)
    return copies


def slot_of(l, F, si):
    """-> (ib, m, chk) consumed by slot si of fill F at layer l."""
    if l == 1:
        f, h = F >> 1, F & 1
        return 0, 0, 4 * (2 * h + si) + f
    if l == 2:
        u, h = F >> 1, F & 1
        return u >> 1, u & 1, 2 * h + si
    s = SPF[l] * F + si
    return s >> 1, s & 1, 0


def rhs_loc(l, ib, chk):
    """-> (tile g, local col) of the rhs slice for slot (ib, *, chk)."""
    colg = ib * LAYER_NPOS[l] + chk * 512
    return colg // TILE_W, colg % TILE_W


# ------------------------------------------------------------------ mirror
def mirror_core(inputs, c):
    """Pure-numpy mirror of the device plan for core c -> fout [4,8,2048]."""
    u0, v0, ly, blocks = core_geom(c)
    w0 = pack_first(inputs["W0"], u0, v0).astype(np.float32)
    pat = pack_patches(inputs["input_data"]).astype(np.float32)
    wl = {l: pack_weights_layer(inputs[f"W{l}"], l, blocks[l], ly)
          .astype(np.float32) for l in range(1, 6)}
    wf = pack_wf(inputs["Wf"], blocks[5]).astype(np.float32)

    act = {l: [[np.zeros((128, TILE_W), np.float32) for _ in range(NTILES[l])]
               for _ in range(2)] for l in range(1, 6)}
    fact = [np.zeros((128, 2048), np.float32) for _ in range(4)]

    def apply(psum, copies, l_next):
        for (sp, scol, dp, p2, g, dcol, sst, dst, nblk, wid) in copies:
            for j in range(nblk):
                if sp < 0:
                    src = psum[:, scol + j * sst:scol + j * sst + wid]
                    dvw = act[l_next][p2][g][:, dcol + j * dst:
                                             dcol + j * dst + wid]
                else:
                    src = psum[sp:sp + 64, scol + j * sst:scol + j * sst + wid]
                    dvw = act[l_next][p2][g][dp:dp + 64,
                                             dcol + j * dst:dcol + j * dst + wid]
                dvw[:] = np.maximum(src, 0.0).astype(BF16).astype(np.float32)

    for F in range(NFIRST):
        psum = np.zeros((128, FILL_W), np.float32)
        for s in range(2):
            psum[:, s * 512:(s + 1) * 512] = (
                w0.T @ pat[:, F * 1024 + s * 512:F * 1024 + (s + 1) * 512])
        apply(psum, first_plan(F), 1)

    for l in range(1, 6):
        K2 = LAYER_K2[l]
        ws = W_SLOT[l]
        for f in range(NFILL):
            psum = np.zeros((128, FILL_W), np.float32)
            for si in range(SPF[l]):
                ib, m, chk = slot_of(l, f, si)
                g, loc = rhs_loc(l, ib, chk)
                grp = wl[l][ib // WGRP[l]]
                base = (ib % WGRP[l]) * 2 * K2 + m * 128
                out = np.zeros((128, ws), np.float32)
                for p in range(2):
                    lhsT = grp[:, base + p * K2:base + p * K2 + 128]
                    out += lhsT.T @ act[l][p][g][:, loc:loc + ws]
                psum[:, si * ws:(si + 1) * ws] = out
            if l == 5:
                fact[f >> 1][:, (f & 1) * 1024:(f & 1) * 1024 + 1024] = (
                    np.maximum(psum, 0.0).astype(BF16).astype(np.float32))
            else:
                apply(psum, layer_plan(l, f), l + 1)

    fout = np.zeros((4, 128, 512), np.float32)
    for fi in range(4):
        for s in range(64):
            idx = 64 * fi + s
            t, dt = s >> 2, s & 3
            rhs = fact[fi][:, s * 32:(s + 1) * 32]
            lhsT = wf[:, idx * 8:idx * 8 + 8]
            fout[fi, 32 * (t & 3) + 8 * dt:32 * (t & 3) + 8 * dt + 8,
                 (t >> 2) * 128 + dt * 32:(t >> 2) * 128 + dt * 32 + 32] = (
                lhsT.T @ rhs)
    return fout


def decode_outputs(fouts):
    out = np.zeros((B, C, 2, 64, 64), np.float32)
    for c, fo in fouts.items():
        _, _, _, blocks = core_geom(c)
        blocks5 = blocks[5]
        rf = np.maximum(np.asarray(fo, np.float32), 0.0)
        for fi in range(4):
            for s in range(64):
                idx = 64 * fi + s
                ib, m = idx // 2, idx % 2
                u, v = blocks5[ib]
                t, dt = s >> 2, s & 3
                r0 = 32 * (t & 3) + 8 * dt
                c0 = (t >> 2) * 128 + dt * 32
                for klx in range(2):
                    U, V = 2 * u + m, 2 * v + klx
                    yf = rf[fi, r0 + klx * 4:r0 + klx * 4 + 4, c0:c0 + 32]
                    out[:, 0, 0, U, V] = yf[0] - yf[2]
                    out[:, 0, 1, U, V] = yf[1] - yf[3]
    return out


def mirror_forward(inputs, cores=range(N_CORES)):
    return decode_outputs({c: mirror_core(inputs, c) for c in cores})


# ------------------------------------------------------------- numpy fallback
def _numpy_reference(inputs):
    x = np.asarray(inputs["input_data"], np.float32)
    b, c_, h, w = x.shape
    xs = np.zeros((b, c_, 4, h, w), np.float32)
    xs[:, :, 0] = x
    p = xs.reshape(b, c_, 4, 32, 2, 32, 2)
    W0 = np.asarray(inputs["W0"], np.float32)
    b0 = np.asarray(inputs["b0"], np.float32)
    y = np.einsum('bcnYhXw,cknhw->bckYX', p, W0) + b0[None, :, :, None, None]
    state = np.maximum(y, 0).reshape(b, c_, 2, 2, NCH, 32, 32)
    for l in range(1, 6):
        Wl = np.asarray(inputs[f"W{l}"], np.float32)
        bl = np.asarray(inputs[f"b{l}"], np.float32)
        G = Wl.shape[1]
        s = state.shape[-1]
        s2 = s // 2
        p = state.reshape(b, c_, G, G, NCH, s2, 2, s2, 2)
        y = np.einsum('bcuvnYpXq,cuvknpq->bcuvkYX', p, Wl) + \
            bl[None, :, :, :, :, None, None]
        y = np.maximum(y, 0).reshape(b, c_, G, G, 2, 2, NCH, s2, s2)
        y = y.transpose(0, 1, 2, 4, 3, 5, 6, 7, 8)
        state = y.reshape(b, c_, 2 * G, 2 * G, NCH, s2, s2)
    st = state.reshape(b, c_, 64, 64, NCH)
    Wf = np.asarray(inputs["Wf"], np.float32)
    bf = np.asarray(inputs["bf"], np.float32)
    yf = np.maximum(np.einsum('bcuvn,cuvkn->bcuvk', st, Wf) + bf[None], 0)
    real = yf[..., 0] - yf[..., 2]
    imag = yf[..., 1] - yf[..., 3]
    return np.stack([real, imag], axis=2)


# ------------------------------------------------------------- bass program
_NC_CACHE = {}


def build_nc(loop=False):
    import concourse.bass as bass
    import concourse.mybir as mybir
    import concourse.tile as tile
    from concourse import bacc
    import contextlib

    F32 = mybir.dt.float32
    BF = mybir.dt.bfloat16
    Relu = mybir.ActivationFunctionType.Relu

    nc = bacc.Bacc(None, target_bir_lowering=False, debug=True)

    d_pat = nc.dram_tensor("patches", [8, 16384], BF, kind="ExternalInput")
    d_w0 = nc.dram_tensor("w0", [8, 128], BF, kind="ExternalInput")
    d_wl = {l: nc.dram_tensor(f"w{l}", [NGRP[l], 128, WGRP[l] * 2 * LAYER_K2[l]],
                              BF, kind="ExternalInput") for l in range(1, 6)}
    d_wf = nc.dram_tensor("wf", [128, 2048], BF, kind="ExternalInput")
    d_out = nc.dram_tensor("fout", [4, 128, 512], F32,
                           kind="ExternalOutput")
    if loop:
        d_bound = nc.dram_tensor("bound", [1, 1], mybir.dt.int32,
                                 kind="ExternalInput")

    with tile.TileContext(nc) as tc:
        with contextlib.ExitStack() as ctx:
            ps = ctx.enter_context(tc.tile_pool(name="ps", bufs=4, space="PSUM"))
            sb = ctx.enter_context(tc.tile_pool(name="sb", bufs=1))
            wpool = ctx.enter_context(tc.tile_pool(name="wp", bufs=1))

            loop_cm = contextlib.nullcontext()
            if loop:
                bt = sb.tile([1, 1], mybir.dt.int32, tag="bt", bufs=1)
                nc.sync.dma_start(out=bt[:], in_=d_bound[:])
                nval = nc.values_load(bt[0:1, 0:1], min_val=0, max_val=1000000,
                                      skip_runtime_bounds_check=True)
                loop_cm = tc.For_i(0, nval, 1)
            ctx.enter_context(loop_cm)

            # ---- first-layer inputs lead the sync queue ----
            w0_sb = sb.tile([8, 128], BF, tag="w0", bufs=1)
            nc.sync.dma_start(out=w0_sb[:], in_=d_w0[:])
            pat_sb = []
            for pi in range(4):
                t = sb.tile([8, 4096], BF, tag="pat", bufs=4, name=f"pat{pi}")
                nc.sync.dma_start(out=t[:], in_=d_pat[:, pi * 4096:
                                                      (pi + 1) * 4096])
                pat_sb.append(t)
            w_sb = {}
            for l in (1, 2, 3):
                t = wpool.tile([128, WGRP[l] * 2 * LAYER_K2[l]], BF,
                               tag=f"w{l}", bufs=1, name=f"w{l}g0")
                nc.sync.dma_start(out=t[:], in_=d_wl[l][0])
                w_sb[l] = [t]
            wf_sb = sb.tile([128, 2048], BF, tag="wf", bufs=1)
            nc.scalar.dma_start(out=wf_sb[:], in_=d_wf[:])

            # ---- big weight stream (sync queue, consumption order) ----
            for l, nbuf in ((4, 4), (5, 7)):
                tiles = []
                for gi in range(NGRP[l]):
                    t = wpool.tile([128, WGRP[l] * 2 * LAYER_K2[l]], BF,
                                   tag=f"w{l}", bufs=nbuf, name=f"w{l}g{gi}")
                    nc.sync.dma_start(out=t[:], in_=d_wl[l][gi])
                    tiles.append(t)
                w_sb[l] = tiles

            # ---- act tiles ----
            act = {l: [[None] * NTILES[l] for _ in range(2)]
                   for l in range(1, 6)}

            def act_tile(l, p, g, eng_tag=None):
                if act[l][p][g] is None:
                    act[l][p][g] = sb.tile([128, TILE_W], BF, tag="act",
                                           bufs=12, name=f"act{l}_{p}{g}")
                return act[l][p][g]

            # ---- relu-scatter emission ----
            # one writer engine per act tile-set: p2=0 -> vector, p2=1 ->
            # scalar. Mixing engines on one tile creates cross-engine WAW
            # semaphore chains that serialize the two copy engines.
            def emit_relu(eng, dst_ap, src_ap):
                if eng is nc.scalar:
                    eng.activation(dst_ap, src_ap, Relu)
                else:
                    eng.tensor_scalar_max(dst_ap, src_ap, 0.0)

            def emit_copies(psum, copies, l_next, whole_eng=None):
                for (sp, scol, dp, p2, g, dcol, sst, dst, nblk, wid) in copies:
                    if sp < 0:
                        # full [128, 2048] copy: single tile, one engine
                        tag = "v" if whole_eng is nc.vector else "s"
                        at = act_tile(l_next, p2, g, eng_tag=tag)
                        emit_relu(whole_eng, at[:, dcol:dcol + wid],
                                  psum[:, scol:scol + wid])
                        continue
                    at = act_tile(l_next, p2, g)
                    eng = nc.vector if p2 == 0 else nc.scalar
                    if nblk == 1:
                        emit_relu(eng,
                                  at[dp:dp + 64, dcol:dcol + wid],
                                  psum[sp:sp + 64, scol:scol + wid])
                    else:
                        src = bass.AP(
                            tensor=psum[:].tensor,
                            offset=psum[:].offset + sp * FILL_W + scol,
                            ap=[[FILL_W, 64], [sst, nblk], [1, wid]])
                        dst_ap = bass.AP(
                            tensor=at[:].tensor,
                            offset=at[:].offset + dp * TILE_W + dcol,
                            ap=[[TILE_W, 64], [dst, nblk], [1, wid]])
                        emit_relu(eng, dst_ap, src)

            # ---- final layer: quads of 4 units per MM (block-diag wf,
            # off-diagonal blocks are computed-and-ignored); raw, relu on host
            def emit_final(fi):
                ft = fact_tiles[fi]
                psF = ps.tile([128, FILL_W], F32, tag="ps", bufs=4,
                              name="psfin")
                for t in range(16):
                    jq, cb = t & 3, t >> 2
                    w0c = (64 * fi + 4 * t) * 8
                    lhsT = wf_sb[:, w0c:w0c + 32]
                    rhs = ft[:, t * 128:(t + 1) * 128]
                    nc.tensor.matmul(psF[32 * jq:32 * jq + 32,
                                         cb * 128:(cb + 1) * 128],
                                     lhsT, rhs, start=True, stop=True,
                                     tile_position=(0, 32 * jq))
                fo = sb.tile([128, 512], F32, tag="fo", bufs=2,
                             name=f"fout{fi}")
                eng = nc.vector if fi % 2 == 0 else nc.scalar
                if eng is nc.scalar:
                    eng.activation(fo[:], psF[:, 0:512],
                                   mybir.ActivationFunctionType.Copy)
                else:
                    eng.tensor_scalar_add(fo[:], psF[:, 0:512], 0.0)
                nc.gpsimd.dma_start(out=d_out[fi], in_=fo[:])

            # ---- first layer: 16 fills ----
            for F in range(NFIRST):
                psum = ps.tile([128, FILL_W], F32, tag="ps", bufs=4, name="psF")
                for s in range(2):
                    c0 = (F % 4) * 1024 + s * 512
                    rhs = pat_sb[F // 4][:, c0:c0 + 512]
                    nc.tensor.matmul(psum[:, s * 512:(s + 1) * 512],
                                     w0_sb[:], rhs, start=True, stop=True)
                emit_copies(psum, first_plan(F), 1,
                            whole_eng=nc.vector if (F >> 1) % 2 == 0
                            else nc.scalar)

            # ---- recursion layers ----
            fact_tiles = []
            for l in range(1, 6):
                K2 = LAYER_K2[l]
                ws = W_SLOT[l]
                for f in range(NFILL):
                    psum = ps.tile([128, FILL_W], F32, tag="ps", bufs=4,
                                   name=f"psl{l}")
                    for si in range(SPF[l]):
                        ib, m, chk = slot_of(l, f, si)
                        g, loc = rhs_loc(l, ib, chk)
                        wt = w_sb[l][ib // WGRP[l]]
                        base = (ib % WGRP[l]) * 2 * K2 + m * 128
                        if l >= 4:
                            # col-tiled M=64 pairs: halve the LDW chain
                            for p in range(2):
                                rhs = act_tile(l, p, g)[:, loc:loc + ws]
                                for cgrp in range(2):
                                    lhsT = wt[:, base + p * K2 + 64 * cgrp:
                                              base + p * K2 + 64 * cgrp + 64]
                                    nc.tensor.matmul(
                                        psum[64 * cgrp:64 * cgrp + 64,
                                             si * ws:(si + 1) * ws],
                                        lhsT, rhs,
                                        start=(p == 0), stop=(p == 1))
                        else:
                            pslice = psum[:, si * ws:(si + 1) * ws]
                            for p in range(2):
                                lhsT = wt[:, base + p * K2:base + p * K2 + 128]
                                rhs = act_tile(l, p, g)[:, loc:loc + ws]
                                nc.tensor.matmul(pslice, lhsT, rhs,
                                                 start=(p == 0), stop=(p == 1))
                    if l == 5:
                        fi, h = f >> 1, f & 1
                        if h == 0:
                            ft = sb.tile([128, 2048], BF, tag="fact", bufs=4,
                                         name=f"fact{fi}")
                            fact_tiles.append(ft)
                        ft = fact_tiles[fi]
                        emit_relu(nc.vector if fi % 2 == 0 else nc.scalar,
                                  ft[:, h * 1024:h * 1024 + 1024], psum[:])
                        if f >= 2 and f % 2 == 0:
                            emit_final((f - 2) // 2)
                        if f == NFILL - 1:
                            emit_final(3)
                    else:
                        emit_copies(psum, layer_plan(l, f), l + 1)
    nc.finalize()
    return nc


# ------------------------------------------------------------------ kernel()
def _pack_in_maps(inputs):
    pat = pack_patches(inputs["input_data"])
    in_maps = []
    for c in range(N_CORES):
        u0, v0, ly, blocks = core_geom(c)
        m = {"patches": pat,
             "w0": pack_first(inputs["W0"], u0, v0),
             "wf": pack_wf(inputs["Wf"], blocks[5])}
        for l in range(1, 6):
            m[f"w{l}"] = pack_weights_layer(inputs[f"W{l}"], l, blocks[l], ly)
        in_maps.append(m)
    return in_maps


def kernel(**inputs):
    exp = {"input_data": (B, C, H, W), "W0": (C, KO, 4, 2, 2), "b0": (C, KO),
           "Wf": (C, 64, 64, 4, NCH), "bf": (C, 64, 64, 4)}
    for l in range(1, 6):
        G = 2 ** l
        exp[f"W{l}"] = (C, G, G, KO, NCH, 2, 2)
        exp[f"b{l}"] = (C, G, G, KO)
    ok = all(tuple(np.shape(inputs.get(k, ()))) == v for k, v in exp.items())
    biases_zero = all(not np.any(np.asarray(inputs[k]))
                      for k in inputs if k.startswith("b"))
    if not ok or not biases_zero:
        return _numpy_reference(inputs)

    from concourse.bass_utils import run_bass_kernel_spmd

    if "nc" not in _NC_CACHE:
        _NC_CACHE["nc"] = build_nc()
    res = run_bass_kernel_spmd(_NC_CACHE["nc"], _pack_in_maps(inputs),
                               core_ids=list(range(N_CORES)))
    return decode_outputs({c: res.results[c]["fout"] for c in range(N_CORES)})
